# revision 1
# baseline (speedup 1.0000x reference)
"""Trainium2 Bass kernel for a quantized shared-expert MLP (SwiGLU, int8 dynamic quant).

Computation (per reference):
  x [2,4096,4096] f32 -> flatten [8192, 4096] -> bf16
  per-token int8 dynamic quant of x; int8 gemm vs w_gate/w_up (per-channel int8);
  swiglu with +-10 clip -> bf16; per-token requant; int8 gemm vs w_down; f32 out.

Strategy: data-parallel over the 8192 tokens across 8 NeuronCores (1024
tokens/core), weights replicated, no collectives.  All matmuls run in bf16,
which is exact here: quantized values are integers in [-127,127] (exact in
bf16) and every partial sum stays below 2^24, so the PE's fp32 accumulation
is bit-exact vs the reference's int gemm.

Per-core layout: token-major tiles [128 tokens, free] for quant/epilogues
(per-token scales are then per-partition [P,1] operands), with DMA-transposes
producing the H-major / I-major quantized operands the PE needs as stationary
lhsT tiles.  Rounding uses the fp32 magic-number trick (x + 1.5*2^23) - 1.5*2^23
== round-to-nearest-even, matching jnp.round exactly.
"""

import numpy as np
import ml_dtypes

H = 4096
I = 2048
P = 128
T = 1024           # tokens per core (8192 / 8)
N_CORES = 8
QMAX = 127.0
LIMIT = 10.0
MAGIC = 12582912.0  # 1.5 * 2**23: fp32 add/sub rounds to nearest-even integer

_CACHE = {}


def _build(tokens=T):
    import concourse.bass as bass
    import concourse.bacc as bacc
    import concourse.mybir as mybir
    from concourse import tile
    from contextlib import ExitStack

    f32 = mybir.dt.float32
    bf16 = mybir.dt.bfloat16
    X = mybir.AxisListType.X
    MAX = mybir.AluOpType.max
    MIN = mybir.AluOpType.min
    MULT = mybir.AluOpType.mult
    ADD = mybir.AluOpType.add
    SUB = mybir.AluOpType.subtract
    Sigmoid = mybir.ActivationFunctionType.Sigmoid

    NT = tokens // P        # token tiles
    KH = H // P             # k-tiles for gate/up gemms (32)
    KI = I // P             # k-tiles for down gemm (16)
    NB = 256                # free-dim block for gate/up gemms
    NB3 = 512               # free-dim block for down gemm
    NBI = I // NB           # 8
    NBH = H // NB3          # 16

    nc = bacc.Bacc("TRN2", target_bir_lowering=False, debug=False)

    x_d = nc.dram_tensor("x", [tokens, H], bf16, kind="ExternalInput")
    wgt_d = nc.dram_tensor("wgt", [H, I], bf16, kind="ExternalInput")
    wut_d = nc.dram_tensor("wut", [H, I], bf16, kind="ExternalInput")
    wdt_d = nc.dram_tensor("wdt", [I, H], bf16, kind="ExternalInput")
    swg_d = nc.dram_tensor("swg", [1, I], f32, kind="ExternalInput")
    swu_d = nc.dram_tensor("swu", [1, I], f32, kind="ExternalInput")
    swd_d = nc.dram_tensor("swd", [1, H], f32, kind="ExternalInput")
    out_d = nc.dram_tensor("out", [tokens, H], f32, kind="ExternalOutput")

    wgt_r = wgt_d.rearrange("(k p) c -> p k c", p=P)
    wut_r = wut_d.rearrange("(k p) c -> p k c", p=P)
    wdt_r = wdt_d.rearrange("(k p) c -> p k c", p=P)

    with ExitStack() as ctx:
        tc = ctx.enter_context(tile.TileContext(nc))

        const_p = ctx.enter_context(tc.tile_pool(name="const", bufs=1))
        sc_p = ctx.enter_context(tc.tile_pool(name="scales", bufs=1))
        inter_p = ctx.enter_context(tc.tile_pool(name="inter", bufs=1))
        # qT lives only through phase 2; closing its stack frees 64KB/partition
        # for the phase-3 weight blocks (allowing 512-wide down-proj tiles).
        qctx = ExitStack()
        qT_p = qctx.enter_context(tc.tile_pool(name="qT", bufs=1))

        swg_b = const_p.tile([P, I], f32, tag="swg_b")
        swu_b = const_p.tile([P, I], f32, tag="swu_b")
        nc.sync.dma_start(swg_b[:], swg_d[0:1, :].broadcast_to([P, I]))
        nc.sync.dma_start(swu_b[:], swu_d[0:1, :].broadcast_to([P, I]))
        zero_b = const_p.tile([P, 1], f32, tag="zero_b")
        nc.vector.memset(zero_b[:], 0.0)

        # per-token-tile scale columns
        mx = sc_p.tile([P, NT], f32, tag="mx")
        sx = sc_p.tile([P, NT], f32, tag="sx")     # x quant scale (= max/127, clamped)
        ix = sc_p.tile([P, NT], f32, tag="ix")     # 1 / sx
        mxi = sc_p.tile([P, NT], f32, tag="mxi")
        si = sc_p.tile([P, NT], f32, tag="si")     # inter quant scale
        ii = sc_p.tile([P, NT], f32, tag="ii")     # 1 / si
        r0 = sc_p.tile([P, NT], f32, tag="r0")     # reciprocal seed / NR temps
        r1 = sc_p.tile([P, NT], f32, tag="r1")

        def nr_recip(out_col, in_col, mc):
            # out = 1/in with one Newton step: r1 = r0*(2 - in*r0); the HW
            # reciprocal seed is not accurate enough for exact round() parity.
            nc.vector.reciprocal(r0[:, mc], in_col)
            nc.vector.tensor_tensor(r1[:, mc], in_col, r0[:, mc], op=MULT)
            nc.vector.tensor_scalar(r1[:, mc], r1[:, mc], -1.0, 2.0, op0=MULT, op1=ADD)
            nc.vector.tensor_tensor(out_col, r0[:, mc], r1[:, mc], op=MULT)

        qT = [qT_p.tile([P, KH, P], bf16, tag=f"qT{m}", name=f"qT{m}")
              for m in range(NT)]
        inter = [inter_p.tile([P, I], bf16, tag=f"inter{m}", name=f"inter{m}") for m in range(NT)]

        # ---- Phase 1: load x token-tiles, dynamic quant, transpose to H-major
        with tc.tile_pool(name="ph1", bufs=2) as ph1:
            for m in range(NT):
                mc = slice(m, m + 1)
                xt = ph1.tile([P, H], bf16, tag="xt", name=f"xt{m}")
                nc.sync.dma_start(xt[:], x_d[m * P:(m + 1) * P, :])
                nc.vector.tensor_reduce(mx[:, mc], xt[:], axis=X, op=MAX,
                                        apply_absolute_value=True)
                nc.vector.tensor_scalar(sx[:, mc], mx[:, mc], 1.0 / QMAX, 1e-8,
                                        op0=MULT, op1=MAX)
                nr_recip(ix[:, mc], sx[:, mc], mc)
                t1 = ph1.tile([P, H], f32, tag="t1", name=f"t1_{m}")
                nc.vector.tensor_scalar(t1[:], xt[:], ix[:, mc], MAGIC, op0=MULT, op1=ADD)
                qt = ph1.tile([P, H], bf16, tag="qt", name=f"qt{m}")
                nc.vector.tensor_scalar(qt[:], t1[:], MAGIC, None, op0=SUB)
                nc.scalar.dma_start(qT[m][:], qt[:], transpose=True)

        # ---- Phase 2: gate/up gemms + swiglu + clip -> inter (bf16)
        with tc.tile_pool(name="wg", bufs=2) as wgp, \
             tc.tile_pool(name="wu", bufs=2) as wup, \
             tc.tile_pool(name="ps2", bufs=3, space=bass.MemorySpace.PSUM) as ps2, \
             tc.tile_pool(name="ep2", bufs=3) as ep2:
            for n in range(NBI):
                nb = slice(n * NB, (n + 1) * NB)
                wgt_t = wgp.tile([P, KH, NB], bf16, tag="wg", name=f"wg{n}")
                wut_t = wup.tile([P, KH, NB], bf16, tag="wu", name=f"wu{n}")
                nc.sync.dma_start(wgt_t[:], wgt_r[:, :, nb])
                nc.sync.dma_start(wut_t[:], wut_r[:, :, nb])
                for m in range(NT):
                    mc = slice(m, m + 1)
                    mb = slice(m * P, (m + 1) * P)
                    pg = ps2.tile([P, NB], f32, tag="pg", name=f"pg{n}_{m}")
                    pu = ps2.tile([P, NB], f32, tag="pu", name=f"pu{n}_{m}")
                    # interleaved so each k's stationary qT block feeds both
                    # the gate and up matmul back-to-back (one weight load)
                    for k in range(KH):
                        nc.tensor.matmul(pg[:], qT[m][:, k, :], wgt_t[:, k, :],
                                         start=(k == 0), stop=(k == KH - 1))
                        nc.tensor.matmul(pu[:], qT[m][:, k, :], wut_t[:, k, :],
                                         start=(k == 0), stop=(k == KH - 1))
                    gs = ep2.tile([P, NB], f32, tag="gs", name=f"gs{n}_{m}")
                    us = ep2.tile([P, NB], f32, tag="us", name=f"us{n}_{m}")
                    nc.vector.scalar_tensor_tensor(gs[:], pg[:], sx[:, mc], swg_b[:, nb],
                                                   op0=MULT, op1=MULT)
                    nc.vector.scalar_tensor_tensor(us[:], pu[:], sx[:, mc], swu_b[:, nb],
                                                   op0=MULT, op1=MULT)
                    sig = ep2.tile([P, NB], f32, tag="sig", name=f"sig{n}_{m}")
                    nc.scalar.activation(sig[:], gs[:], Sigmoid, bias=zero_b[:])
                    slu = ep2.tile([P, NB], f32, tag="slu", name=f"slu{n}_{m}")
                    nc.vector.tensor_tensor(slu[:], sig[:], gs[:], op=MULT)
                    pr = ep2.tile([P, NB], f32, tag="pr", name=f"pr{n}_{m}")
                    nc.vector.tensor_tensor(pr[:], slu[:], us[:], op=MULT)
                    nc.vector.tensor_scalar(inter[m][:, nb], pr[:], LIMIT, -LIMIT,
                                            op0=MIN, op1=MAX)

        qctx.close()

        # ---- Phase 2.5: requant inter, transpose to I-major
        qiT_p = ctx.enter_context(tc.tile_pool(name="qiT", bufs=1))
        qiT = [qiT_p.tile([P, KI, P], bf16, tag=f"qiT{m}", name=f"qiT{m}")
               for m in range(NT)]
        with tc.tile_pool(name="rq", bufs=2) as rq:
            for m in range(NT):
                mc = slice(m, m + 1)
                nc.vector.tensor_reduce(mxi[:, mc], inter[m][:], axis=X, op=MAX,
                                        apply_absolute_value=True)
                nc.vector.tensor_scalar(si[:, mc], mxi[:, mc], 1.0 / QMAX, 1e-8,
                                        op0=MULT, op1=MAX)
                nr_recip(ii[:, mc], si[:, mc], mc)
                t2 = rq.tile([P, I], f32, tag="t2", name=f"t2_{m}")
                nc.vector.tensor_scalar(t2[:], inter[m][:], ii[:, mc], MAGIC,
                                        op0=MULT, op1=ADD)
                qi = rq.tile([P, I], bf16, tag="qi", name=f"qi{m}")
                nc.vector.tensor_scalar(qi[:], t2[:], MAGIC, None, op0=SUB)
                nc.scalar.dma_start(qiT[m][:], qi[:], transpose=True)

        # ---- Phase 3: down gemm + scales -> out
        with tc.tile_pool(name="wd", bufs=2) as wdp, \
             tc.tile_pool(name="swd", bufs=1) as swdp, \
             tc.tile_pool(name="ps3", bufs=3, space=bass.MemorySpace.PSUM) as ps3, \
             tc.tile_pool(name="outp", bufs=6) as outp:
            swd_b = swdp.tile([P, H], f32, tag="swd_b")
            nc.sync.dma_start(swd_b[:], swd_d[0:1, :].broadcast_to([P, H]))
            for n in range(NBH):
                nb = slice(n * NB3, (n + 1) * NB3)
                wdt_t = wdp.tile([P, KI, NB3], bf16, tag="wd", name=f"wd{n}")
                nc.sync.dma_start(wdt_t[:], wdt_r[:, :, nb])
                for m in range(NT):
                    mc = slice(m, m + 1)
                    mb = slice(m * P, (m + 1) * P)
                    po = ps3.tile([P, NB3], f32, tag="po", name=f"po{n}_{m}")
                    for k in range(KI):
                        nc.tensor.matmul(po[:], qiT[m][:, k, :], wdt_t[:, k, :],
                                         start=(k == 0), stop=(k == KI - 1))
                    ot = outp.tile([P, NB3], f32, tag="ot", name=f"ot{n}_{m}")
                    nc.vector.scalar_tensor_tensor(ot[:], po[:], si[:, mc], swd_b[:, nb],
                                                   op0=MULT, op1=MULT)
                    nc.sync.dma_start(out_d[m * P:(m + 1) * P, nb], ot[:])

    if not nc.is_finalized():
        nc.finalize()
    return nc


def _prep_inputs(x, w_gate, s_wgate, w_up, s_wup, w_down, s_wdown):
    bf16 = ml_dtypes.bfloat16
    x_flat = np.ascontiguousarray(x.reshape(-1, H)).astype(bf16)
    wgt = np.ascontiguousarray(w_gate.astype(bf16).T)   # int-valued: cast exact
    wut = np.ascontiguousarray(w_up.astype(bf16).T)
    wdt = np.ascontiguousarray(w_down.astype(bf16).T)
    swg = np.ascontiguousarray(s_wgate.reshape(1, I).astype(np.float32))
    swu = np.ascontiguousarray(s_wup.reshape(1, I).astype(np.float32))
    swd = np.ascontiguousarray(s_wdown.reshape(1, H).astype(np.float32))
    return x_flat, wgt, wut, wdt, swg, swu, swd


def kernel(x, w_gate, s_wgate, w_up, s_wup, w_down, s_wdown,
           inv_gate, inv_up, inv_inter):
    from concourse.bass_utils import run_bass_kernel_spmd

    x_flat, wgt, wut, wdt, swg, swu, swd = _prep_inputs(
        x, w_gate, s_wgate, w_up, s_wup, w_down, s_wdown)

    if "nc" not in _CACHE:
        _CACHE["nc"] = _build()
    nc = _CACHE["nc"]

    in_maps = []
    for c in range(N_CORES):
        in_maps.append({
            "x": np.ascontiguousarray(x_flat[c * T:(c + 1) * T]),
            "wgt": wgt, "wut": wut, "wdt": wdt,
            "swg": swg, "swu": swu, "swd": swd,
        })
    res = run_bass_kernel_spmd(nc, in_maps, list(range(N_CORES)))
    _CACHE["last_results"] = res
    _CACHE["last_in_maps"] = in_maps
    out = np.concatenate([res.results[c]["out"] for c in range(N_CORES)], axis=0)
    return out.reshape(x.shape).astype(np.float32)

